# revision 1
# baseline (speedup 1.0000x reference)
"""ALiBi bidirectional attention — 8-core Trainium2 Bass kernel.

Problem: B=2, T=2048, D=1024, H=16, hd=64, f32 in/out.
reference: softmax(Q K^T/8 + slopes_h * -|i-j|) V, then out-proj.

Sharding (sequence-parallel): core c handles batch c//4 and query rows
q0 = 512*(c%4) .. q0+512. Out-proj contracts the full model dim locally,
so the output is a pure concat of per-core [512, 1024] slices.

K^T is projected on the owning slice and AllGathered within the 4-core
batch group (1MB bf16, mesh) — the only collective. V is recomputed in
full on every core (the extra ~4.3 GFLOP of matmul overlaps the K
AllGather instead of paying a second ~60us serial collective).

SPMD rotation: k-position data lives in per-core LOCAL coordinates
k_local = (k_phys - q0) mod 2048, so the diagonal-crossing band is
always local tiles kt 0..3 and the graph is identical on every core.
V's rotation is a host-side np.roll of the transposed input; K's
rotation happens in the per-head gather-back DMAs whose source block
index comes from a host-passed table via register-offset APs.

ALiBi: with s = bf16-snapped slope and diff = k_phys - q_phys:
  * non-crossing k-tiles: bias = -s|diff| is affine per tile; the exp's
    [P,1] bias carries -+s*p (f32); two extra contract rows in the
    scores matmul carry the q_lo part and the per-tile constant, with
    exactly-representable bf16 factors (ints x s / 128s).
  * crossing tiles (kt 0..3): scores exp'd raw, then multiplied by
    exp(-s|diff|) from a shifted-window table EW[p,col]=exp(-s|p-col+384|).
Scores are computed transposed (ST = [kpos, q]) so probs feed the AV
matmul as lhsT-ready; a ones column in V yields softmax row-sums in the
same matmul; no row-max pass (args <= ~6, exp cannot overflow).
(h, kt) tiles where s_h*min|diff| >= 115 underflow to exactly 0.0 in
f32 on every core and are skipped entirely (bitwise-identical result).

The attention is split into an own-block pass (local kt 0..3, needs no
gathered data — overlaps the AllGather) and a rest pass (kt 4..15) that
adds the own-block partial O^T back before normalizing.
"""
import math
import sys

sys.path.insert(0, "/opt/trn_rl_repo")

import numpy as np

from concourse import bass, bacc
import concourse.tile as tile
from concourse.bass_utils import run_bass_kernel_spmd

mybir = bass.mybir
FP32 = mybir.dt.float32
BF16 = mybir.dt.bfloat16
INT32 = mybir.dt.int32

B, T, D = 2, 2048, 1024
H, HD = 16, 64
NCORES = 8
QS = 512                      # query rows per core
NKT = T // 128                # 16 k tiles
GROUPS = [[0, 1, 2, 3], [4, 5, 6, 7]]

try:
    import ml_dtypes
    BF16_NP = np.dtype(ml_dtypes.bfloat16)
except ImportError:
    BF16_NP = None


def _bf16_round_f32(x):
    u = np.asarray(x, np.float32).view(np.uint32)
    r = (u + 0x7FFF + ((u >> 16) & 1)) & 0xFFFF0000
    return r.astype(np.uint32).view(np.float32)


def _slopes():
    start = 2.0 ** (-(2.0 ** (-(math.log2(H) - 3))))
    return np.asarray([start * start ** i for i in range(H)], np.float32)


SLOPES = _bf16_round_f32(_slopes())     # used consistently everywhere


def _skippable(h, kt):
    # exp(score - s|diff|) underflows f32 to exactly 0 on every core
    m = min(128 * kt - 511, 1921 - 128 * kt)
    return SLOPES[h] * m >= 115.0


SKIP_GROUPS = [
    {g for g in range(6)
     if _skippable(h, 4 + 2 * g) and _skippable(h, 5 + 2 * g)}
    for h in range(H)
]

# --------------------------------------------------------------------------
# graph
# --------------------------------------------------------------------------


def _build_graph():
    nc = bacc.Bacc("TRN2", target_bir_lowering=False, debug=False,
                   num_devices=NCORES)

    p = {}
    p["xq"] = nc.declare_dram_parameter("xq", [D, QS], BF16, isOutput=False)
    p["xk"] = nc.declare_dram_parameter("xk", [D, QS], BF16, isOutput=False)
    p["xv"] = nc.declare_dram_parameter("xv", [D, T], BF16, isOutput=False)
    for nm in ("wq", "wk", "wv", "wo"):
        p[nm] = nc.declare_dram_parameter(nm, [D, D], BF16, isOutput=False)
    p["qlo"] = nc.declare_dram_parameter("qlo", [2, H, QS], BF16, isOutput=False)
    p["srow"] = nc.declare_dram_parameter("srow", [H, 2, T], BF16, isOutput=False)
    p["biasall"] = nc.declare_dram_parameter("biasall", [128, H * 8], FP32,
                                             isOutput=False)
    p["ew"] = nc.declare_dram_parameter("ew", [128, H, 896], BF16,
                                        isOutput=False)
    p["rotidx"] = nc.declare_dram_parameter("rotidx", [1, 4], INT32,
                                            isOutput=False)
    p["out"] = nc.declare_dram_parameter("out", [QS, D], FP32, isOutput=True)

    bounce_k = nc.dram_tensor("bounce_k", [D, QS], BF16)
    agk = nc.dram_tensor("agk", [4, D, QS], BF16)

    with tile.TileContext(nc) as tc:
        _emit(tc, nc, p, bounce_k, agk)

    nc.compile()
    return nc


def _emit(tc, nc, p, bounce_k, agk):
    Exp = mybir.ActivationFunctionType.Exp
    import contextlib
    ctx = contextlib.ExitStack()

    cpool = ctx.enter_context(tc.tile_pool(name="consts", bufs=1))
    kvq = ctx.enter_context(tc.tile_pool(name="kvq", bufs=1))
    late = ctx.enter_context(tc.tile_pool(name="late", bufs=1))

    # long-lived tensors (DMAs for inputs emitted in need-order)
    rot_sb = cpool.tile([1, 4], INT32)
    nc.sync.dma_start(rot_sb[:], p["rotidx"].ap())
    qt = kvq.tile([66, H, QS], BF16)            # Q^T (+qlo, +const rows)
    klocal = kvq.tile([64, H, QS], BF16)        # own K^T slice, head-major
    vfull = kvq.tile([128, NKT, H, 65], BF16)   # full V, local coords (+ones)
    ot_own = kvq.tile([65, H, QS], BF16)        # own-block partial O^T
    ot = kvq.tile([128, 8, QS], BF16)           # normalized O^T

    # ================= phase 1: projections ===============================
    with tc.tile_pool(name="xw", bufs=1) as xw, \
         tc.tile_pool(name="wrot", bufs=2) as wrot, \
         tc.tile_pool(name="projps", bufs=3, space="PSUM") as pp:

        def load_x(nm, width):
            tag = "xs" if nm in ("xk", "xq") else nm
            t = xw.tile([128, 8, width], BF16, tag=tag)
            nc.sync.dma_start(t[:], p[nm].ap().rearrange(
                "(j p) c -> p j c", p=128))
            return t

        def load_w(nm):
            t = wrot.tile([128, 8, D], BF16, tag="w")
            nc.sync.dma_start(t[:], p[nm].ap().rearrange(
                "(j p) c -> p j c", p=128))
            return t

        # ---- K projection (own slice) + send + AllGather, ASAP ----
        xk = load_x("xk", QS)
        wk_sb = load_w("wk")
        for j in range(8):
            ps = pp.tile([128, QS], FP32, tag="proj")
            for cj in range(8):
                nc.tensor.matmul(ps[:], wk_sb[:, cj, 128 * j:128 * (j + 1)],
                                 xk[:, cj, :], start=(cj == 0), stop=(cj == 7))
            nc.vector.tensor_copy(klocal[:, 2 * j, :], ps[0:64, :])
            nc.vector.tensor_copy(klocal[:, 2 * j + 1, :], ps[64:128, :])
        nc.sync.dma_start(
            bounce_k.ap().rearrange("(h p) c -> p h c", p=64), klocal[:])
        nc.gpsimd.collective_compute(
            "AllGather", mybir.AluOpType.bypass, replica_groups=GROUPS,
            ins=[bounce_k.ap().opt()], outs=[agk.ap().opt()])

        # ---- V projection: FULL batch, local coords (overlaps the AG) ----
        xv = load_x("xv", T)
        wv_sb = load_w("wv")
        nc.vector.memset(vfull[:, :, :, 64:65], 1.0)
        for tc_i in range(NKT):
            for half in range(2):
                ps = pp.tile([128, 512], FP32, tag="proj")
                for cj in range(8):
                    nc.tensor.matmul(ps[:],
                                     xv[:, cj, 128 * tc_i:128 * (tc_i + 1)],
                                     wv_sb[:, cj, 512 * half:512 * (half + 1)],
                                     start=(cj == 0), stop=(cj == 7))
                nc.vector.tensor_copy(
                    vfull[:, tc_i, 8 * half:8 * (half + 1), 0:64],
                    ps[:].rearrange("p (h d) -> p h d", h=8))

        # ---- Q projection ----
        xq = load_x("xq", QS)
        wq_sb = load_w("wq")
        for j in range(8):
            ps = pp.tile([128, QS], FP32, tag="proj")
            for cj in range(8):
                nc.tensor.matmul(ps[:], wq_sb[:, cj, 128 * j:128 * (j + 1)],
                                 xq[:, cj, :], start=(cj == 0), stop=(cj == 7))
            nc.vector.tensor_copy(qt[0:64, 2 * j, :], ps[0:64, :])
            nc.vector.tensor_copy(qt[0:64, 2 * j + 1, :], ps[64:128, :])
        nc.sync.dma_start(qt[64:66, :, :], p["qlo"].ap())

    # consts needed from the own-block pass on
    biasall = cpool.tile([128, H * 8], FP32)
    nc.sync.dma_start(biasall[:], p["biasall"].ap())
    ew = cpool.tile([128, H, 896], BF16)
    nc.sync.dma_start(ew[:], p["ew"].ap())

    # ================= phase 2: attention =================================
    with tc.tile_pool(name="ktstream", bufs=3) as kts, \
         tc.tile_pool(name="exps", bufs=4) as epool, \
         tc.tile_pool(name="recip", bufs=3) as rpool, \
         tc.tile_pool(name="yout", bufs=2) as ypool, \
         tc.tile_pool(name="stps", bufs=3, space="PSUM") as stp, \
         tc.tile_pool(name="otps", bufs=2, space="PSUM") as otp:

        # --- own-block pass: local kt 0..3 (no gathered data needed) ---
        for h in range(H):
            oo = otp.tile([65, QS], FP32, tag="ot")
            for g in range(2):
                stps = stp.tile([128, 2 * QS], FP32, tag="st")
                for j in range(2):
                    kt = 2 * g + j
                    nc.tensor.matmul(stps[:, QS * j:QS * (j + 1)],
                                     klocal[:, h, 128 * kt:128 * (kt + 1)],
                                     qt[0:64, h, :], start=True, stop=True)
                e = epool.tile([128, 2 * QS], BF16, tag="e")
                nc.scalar.activation(e[:], stps[:], Exp,
                                     bias=biasall[:, 8 * h:8 * h + 1],
                                     scale=1.0)
                for j in range(2):
                    kt = 2 * g + j
                    nc.vector.tensor_mul(e[:, QS * j:QS * (j + 1)],
                                         e[:, QS * j:QS * (j + 1)],
                                         ew[:, h, 384 - 128 * kt:896 - 128 * kt])
                    nc.tensor.matmul(oo[:], vfull[:, kt, h, :],
                                     e[:, QS * j:QS * (j + 1)],
                                     start=(kt == 0), stop=(kt == 3))
            nc.vector.tensor_copy(ot_own[:, h, :], oo[:])

        # --- per-core K rotation registers ---
        rvs = []
        for rl in range(4):
            reg = nc.sync.alloc_register(f"rot{rl}")
            nc.sync.reg_load(reg, rot_sb[0:1, rl:rl + 1])
            rvs.append(nc.sync.snap(reg, donate=True))
        agk_r = agk.ap().rearrange("r (h d) c -> r d h c", h=H)    # [4,64,H,QS]

        # --- rest pass: kt 4..15 from gathered K, then finalize ---
        for h in range(H):
            kept = [g for g in range(6) if g not in SKIP_GROUPS[h]]
            kth = kts.tile([66, 3 * QS], BF16, tag="kth")
            for rl in range(1, 4):
                if all((kt - 4) // 2 in SKIP_GROUPS[h]
                       for kt in range(4 * rl, 4 * rl + 4)):
                    continue    # whole block underflows to zero
                nc.sync.dma_start(kth[0:64, QS * (rl - 1):QS * rl],
                                  agk_r[bass.ds(rvs[rl], 1), :, h, :])
            nc.sync.dma_start(kth[64:66, :], p["srow"].ap()[h, :, QS:])

            otps = otp.tile([65, QS], FP32, tag="ot")
            for g in kept:
                stps = stp.tile([128, 2 * QS], FP32, tag="st")
                for j in range(2):
                    kt = 4 + 2 * g + j
                    nc.tensor.matmul(stps[:, QS * j:QS * (j + 1)],
                                     kth[:, 128 * (kt - 4):128 * (kt - 3)],
                                     qt[:, h, :], start=True, stop=True)
                e = epool.tile([128, 2 * QS], BF16, tag="e")
                nc.scalar.activation(e[:], stps[:], Exp,
                                     bias=biasall[:, 8 * h + 1 + g:8 * h + 2 + g],
                                     scale=1.0)
                for j in range(2):
                    kt = 4 + 2 * g + j
                    nc.tensor.matmul(otps[:], vfull[:, kt, h, :],
                                     e[:, QS * j:QS * (j + 1)],
                                     start=(g == kept[0] and j == 0),
                                     stop=(g == kept[-1] and j == 1))
            nc.vector.tensor_add(otps[:], otps[:], ot_own[:, h, :])
            rec = rpool.tile([1, QS], FP32, tag="rec")
            nc.vector.reciprocal(rec[:], otps[64:65, :])
            bcs = rpool.tile([64, QS], FP32, tag="bcs")
            nc.gpsimd.partition_broadcast(bcs[:], rec[:])
            nc.vector.tensor_mul(ot[64 * (h % 2):64 * (h % 2) + 64, h // 2, :],
                                 otps[0:64, :], bcs[:])

        # --- out-projection (wo loaded late, slot materializes here) ---
        wo_sb = late.tile([128, 8, D], BF16)
        nc.sync.dma_start(wo_sb[:], p["wo"].ap().rearrange(
            "(j p) c -> p j c", p=128))
        for tc_i in range(4):
            y = ypool.tile([128, D], FP32, tag="y")
            for nh in range(2):
                ps = otp.tile([128, 512], FP32, tag="ot")
                for j in range(8):
                    nc.tensor.matmul(ps[:], ot[:, j, 128 * tc_i:128 * (tc_i + 1)],
                                     wo_sb[:, j, 512 * nh:512 * (nh + 1)],
                                     start=(j == 0), stop=(j == 7))
                nc.vector.tensor_copy(y[:, 512 * nh:512 * (nh + 1)], ps[:])
            nc.sync.dma_start(p["out"].ap()[128 * tc_i:128 * (tc_i + 1), :], y[:])

    ctx.close()


# --------------------------------------------------------------------------
# host side
# --------------------------------------------------------------------------

def _prep_core_inputs(inputs, c):
    b, s = divmod(c, 4)
    q0 = QS * s
    sl = slice(q0, q0 + QS)
    f32 = np.float32

    for bn in ("bq", "bk", "bv", "bo"):
        assert not np.any(np.asarray(inputs[bn])), \
            f"nonzero {bn} not supported by this kernel build"

    def tr(x):
        return np.ascontiguousarray(np.asarray(x, f32).T)

    xv_rot = np.roll(tr(inputs["value"][b]), -q0, axis=1)  # local coords
    m = {
        "xq": tr(inputs["query"][b][sl]).astype(BF16_NP),
        "xk": tr(inputs["key"][b][sl]).astype(BF16_NP),
        "xv": xv_rot.astype(BF16_NP),
        "wq": (np.asarray(inputs["Wq"], f32) * HD ** -0.5).astype(BF16_NP),
        "wk": np.asarray(inputs["Wk"], f32).astype(BF16_NP),
        "wv": np.asarray(inputs["Wv"], f32).astype(BF16_NP),
        "wo": np.asarray(inputs["Wo"], f32).astype(BF16_NP),
    }

    qlo = np.zeros((2, H, QS), f32)
    qlo[0] = (np.arange(QS, dtype=f32) - 256.0)[None, :]
    qlo[1] = (128.0 * SLOPES)[:, None]
    m["qlo"] = qlo.astype(BF16_NP)

    # local k coords; wrap where k_local >= T - q0 (512-aligned)
    kloc = np.arange(T)
    wrap = kloc >= (T - q0) if q0 > 0 else np.zeros(T, bool)
    ktv = kloc // 128
    srow = np.zeros((H, 2, T), f32)
    biasall = np.zeros((128, H, 8), f32)
    pvec = np.arange(128, dtype=f32)
    for h in range(H):
        sh = SLOPES[h]
        # row 0: coefficient of (q_lo - 256); row 1: coefficient of 128*s
        srow[h, 0, 512:] = np.where(wrap[512:], -sh, sh)
        srow[h, 1, 512:] = np.where(wrap[512:], ktv[512:] - 18.0,
                                    2.0 - ktv[512:])
        for g in range(6):
            kt = 4 + 2 * g
            biasall[:, h, 1 + g] = (sh * pvec) if wrap[128 * kt] else (-sh * pvec)
    m["srow"] = srow.astype(BF16_NP)
    m["biasall"] = biasall.reshape(128, H * 8)

    col = np.arange(896, dtype=f32)
    x = pvec[:, None] - col[None, :] + 384.0            # [128, 896]
    ewf = np.exp(-np.abs(x)[:, None, :] * SLOPES[None, :, None])
    m["ew"] = ewf.astype(BF16_NP)

    m["rotidx"] = np.asarray([[(rl + s) % 4 for rl in range(4)]], np.int32)
    return m


_NC_CACHE = {}


def _get_nc():
    if "nc" not in _NC_CACHE:
        _NC_CACHE["nc"] = _build_graph()
    return _NC_CACHE["nc"]


def run(inputs, trace=False, trace_kwargs=None):
    nc = _get_nc()
    in_maps = [_prep_core_inputs(inputs, c) for c in range(NCORES)]
    res = run_bass_kernel_spmd(nc, in_maps, list(range(NCORES)),
                               trace=trace, **(trace_kwargs or {}))
    out = np.empty((B, T, D), np.float32)
    for c in range(NCORES):
        b, s = divmod(c, 4)
        out[b, QS * s:QS * (s + 1), :] = res.results[c]["out"]
    return out, res


def kernel(**inputs):
    return run(inputs)[0]



# revision 15
# speedup vs baseline: 1.1103x; 1.1103x over previous
"""ALiBi bidirectional attention — 8-core Trainium2 Bass kernel.

Problem: B=2, T=2048, D=1024, H=16, hd=64, f32 in/out.
reference: softmax(Q K^T/8 + slopes_h * -|i-j|) V, then out-proj.

Sharding (sequence-parallel): core c handles batch c//4 and query rows
q0 = 512*(c%4) .. q0+512. Out-proj contracts the full model dim locally,
so the output is a pure concat of per-core [512, 1024] slices.

K^T is projected on the owning slice and AllGathered within the 4-core
batch group (1MB bf16, mesh) — the only collective. V is recomputed in
full on every core (the extra matmul work overlaps the K AllGather).

SPMD rotation: k-position data lives in per-core LOCAL coordinates
k_local = (k_phys - q0) mod 2048, so the diagonal-crossing band is
always local tiles kt 0..3 and the graph is identical on every core.
V's rotation is a host-side np.roll of the transposed input; K's
rotation happens in the per-head gather-back DMAs whose source block
index comes from a host-passed table via register-offset APs.

ALiBi: with s = bf16-snapped slope and diff = k_phys - q_phys:
  * non-crossing k-tiles: bias = -s|diff| is affine per tile; the exp's
    [P,1] bias carries -+s*p (f32); two extra contract rows in the
    scores matmul carry the q_lo part and the per-tile constant.
  * crossing tiles (kt 0..3): scores exp'd raw, then multiplied by
    exp(-s|diff|) from a shifted-window table EW[p,col]=exp(-s|p-col+384|)
    generated on-chip (|p-col+384| base DMA'd, per-head scalar-engine Exp).
Scores are computed transposed (ST = [kpos, q]) so probs feed the AV
matmul as lhsT-ready; a ones column in V yields softmax row-sums in the
same matmul; no row-max pass (args <= ~6, exp cannot overflow).
(h, kt) tiles where s_h*min|diff| >= 25 contribute relative attention
mass < ~1e-4 and are skipped entirely.

Schedule (per core): K-proj (cj-pipelined input DMAs) -> AllGather
launched; Q-proj; EW gen; V-proj of the own 4 k-tiles; then the
own-block attention pass (local kt 0..3) interleaved tile-by-tile with
the remaining 12 V-proj k-tiles so the PE stays dense while exp/DVE
chains fill in behind it; then the rest pass (kt 4..15, heavy heads
first) from gathered K; out-proj (N=1024 matmuls).
"""
import math
import sys

sys.path.insert(0, "/opt/trn_rl_repo")

import numpy as np

from concourse import bass, bacc
import concourse.tile as tile
from concourse.bass_utils import run_bass_kernel_spmd

mybir = bass.mybir
FP32 = mybir.dt.float32
BF16 = mybir.dt.bfloat16
INT32 = mybir.dt.int32

B, T, D = 2, 2048, 1024
H, HD = 16, 64
NCORES = 8
QS = 512                      # query rows per core
NKT = T // 128                # 16 k tiles
GROUPS = [[0, 1, 2, 3], [4, 5, 6, 7]]

try:
    import ml_dtypes
    BF16_NP = np.dtype(ml_dtypes.bfloat16)
except ImportError:
    BF16_NP = None

DEBUG_DUMP = False


def _bf16_round_f32(x):
    u = np.asarray(x, np.float32).view(np.uint32)
    r = (u + 0x7FFF + ((u >> 16) & 1)) & 0xFFFF0000
    return r.astype(np.uint32).view(np.float32)


def _slopes():
    start = 2.0 ** (-(2.0 ** (-(math.log2(H) - 3))))
    return np.asarray([start * start ** i for i in range(H)], np.float32)


SLOPES = _bf16_round_f32(_slopes())     # used consistently everywhere

SKIP_THRESH = 25.0


def _skippable(h, kt):
    # dropped attention mass is < ~e^-(25-ln(n)-score spread): negligible
    m = min(128 * kt - 511, 1921 - 128 * kt)
    return SLOPES[h] * m >= SKIP_THRESH


SKIP_GROUPS = [
    {g for g in range(6)
     if _skippable(h, 4 + 2 * g) and _skippable(h, 5 + 2 * g)}
    for h in range(H)
]
# rest pass: heavy heads first so the post-softmax tail is short
HEAD_ORDER = sorted(range(H), key=lambda h: len(SKIP_GROUPS[h]))

# --------------------------------------------------------------------------
# graph
# --------------------------------------------------------------------------


def _build_graph():
    nc = bacc.Bacc("TRN2", target_bir_lowering=False, debug=False,
                   num_devices=NCORES)

    p = {}
    p["xq"] = nc.declare_dram_parameter("xq", [D, QS], BF16, isOutput=False)
    p["xk"] = nc.declare_dram_parameter("xk", [D, QS], BF16, isOutput=False)
    p["xv"] = nc.declare_dram_parameter("xv", [D, T], BF16, isOutput=False)
    for nm in ("wq", "wk", "wv", "wo"):
        p[nm] = nc.declare_dram_parameter(nm, [D, D], BF16, isOutput=False)
    p["qlo"] = nc.declare_dram_parameter("qlo", [2, H, QS], BF16, isOutput=False)
    p["srow"] = nc.declare_dram_parameter("srow", [H, 2, T], BF16, isOutput=False)
    p["biasall"] = nc.declare_dram_parameter("biasall", [128, H * 8], FP32,
                                             isOutput=False)
    p["dbase"] = nc.declare_dram_parameter("dbase", [128, 896], FP32,
                                           isOutput=False)
    p["rotidx"] = nc.declare_dram_parameter("rotidx", [1, 4], INT32,
                                            isOutput=False)
    p["out"] = nc.declare_dram_parameter("out", [QS, D], FP32, isOutput=True)
    if DEBUG_DUMP:
        p["d_klocal"] = nc.declare_dram_parameter("d_klocal", [64, H, QS], BF16, isOutput=True)
        p["d_qt"] = nc.declare_dram_parameter("d_qt", [66, H, QS], BF16, isOutput=True)
        p["d_vfull"] = nc.declare_dram_parameter("d_vfull", [128, NKT, H, 65], BF16, isOutput=True)
        p["d_otown"] = nc.declare_dram_parameter("d_otown", [65, H, QS], BF16, isOutput=True)
        p["d_ew"] = nc.declare_dram_parameter("d_ew", [128, H, 896], BF16, isOutput=True)
        p["d_ot"] = nc.declare_dram_parameter("d_ot", [128, 8, QS], BF16, isOutput=True)
        p["d_rec"] = nc.declare_dram_parameter("d_rec", [1, H, QS], FP32, isOutput=True)
        p["d_z"] = nc.declare_dram_parameter("d_z", [1, H, QS], FP32, isOutput=True)

    bounce_k = nc.dram_tensor("bounce_k", [D, QS], BF16)
    agk = nc.dram_tensor("agk", [4, D, QS], BF16)

    with tile.TileContext(nc) as tc:
        _emit(tc, nc, p, bounce_k, agk)

    nc.compile()
    return nc


def _emit(tc, nc, p, bounce_k, agk):
    Exp = mybir.ActivationFunctionType.Exp
    import contextlib
    ctx = contextlib.ExitStack()

    cpool = ctx.enter_context(tc.tile_pool(name="consts", bufs=1))
    kvq = ctx.enter_context(tc.tile_pool(name="kvq", bufs=1))
    late = ctx.enter_context(tc.tile_pool(name="late", bufs=1))

    rot_sb = cpool.tile([1, 4], INT32)
    nc.sync.dma_start(rot_sb[:], p["rotidx"].ap())
    qt = kvq.tile([66, H, QS], BF16)            # Q^T (+qlo, +const rows)
    klocal = kvq.tile([64, H, QS], BF16)        # own K^T slice, head-major
    vfull = kvq.tile([128, NKT, H, 65], BF16)   # full V, local coords (+ones)
    ot_own = kvq.tile([65, H, QS], BF16)        # own-block partial O^T
    ew = cpool.tile([128, H, 896], BF16)        # crossing-tile exp windows
    biasall = cpool.tile([128, H * 8], FP32)

    def cast(idx, dst, src):
        # alternate psum->sbuf casts across the two free engines
        if idx % 2 == 0:
            nc.scalar.copy(dst, src)
        else:
            nc.vector.tensor_copy(dst, src)

    # ================= phase 1: projections + own-block ===================
    pctx = contextlib.ExitStack()
    xw = pctx.enter_context(tc.tile_pool(name="xw", bufs=1))
    wrot = pctx.enter_context(tc.tile_pool(name="wrot", bufs=2))

    def load_split(xt, wt, xnm, wnm):
        # per-cj interleaved input DMAs so compute starts on chunk 0
        xsrc = p[xnm].ap().rearrange("(j p) c -> p j c", p=128)
        wsrc = p[wnm].ap().rearrange("(j p) c -> p j c", p=128)
        for cj in range(8):
            nc.sync.dma_start(xt[:, cj, :], xsrc[:, cj, :])
            nc.sync.dma_start(wt[:, cj, :], wsrc[:, cj, :])

    with tc.tile_pool(name="pp8", bufs=1, space="PSUM") as pp8:
        # ---- K projection (own slice), cj-outer over 8 live psum banks ---
        xk = xw.tile([128, 8, QS], BF16, tag="xk")
        wk_sb = wrot.tile([128, 8, D], BF16, tag="w")
        load_split(xk, wk_sb, "xk", "wk")
        psk = [pp8.tile([128, QS], FP32, tag=f"p{j}", name=f"psk{j}")
               for j in range(8)]
        for cj in range(8):
            for j in range(8):
                nc.tensor.matmul(psk[j][:], wk_sb[:, cj, 128 * j:128 * (j + 1)],
                                 xk[:, cj, :], start=(cj == 0), stop=(cj == 7))
        for j in range(8):
            cast(0, klocal[:, 2 * j, :], psk[j][0:64, :])
            cast(1, klocal[:, 2 * j + 1, :], psk[j][64:128, :])
        nc.sync.dma_start(
            bounce_k.ap().rearrange("(h p) c -> p h c", p=64), klocal[:])
        nc.gpsimd.collective_compute(
            "AllGather", mybir.AluOpType.bypass, replica_groups=GROUPS,
            ins=[bounce_k.ap().opt()], outs=[agk.ap().opt()])

        # ---- Q projection, same shape (xq reuses xk's slot) ----
        xq = xw.tile([128, 8, QS], BF16, tag="xk")
        wq_sb = wrot.tile([128, 8, D], BF16, tag="w")
        load_split(xq, wq_sb, "xq", "wq")
        nc.sync.dma_start(qt[64:66, :, :], p["qlo"].ap())
        psq = [pp8.tile([128, QS], FP32, tag=f"p{j}", name=f"psq{j}")
               for j in range(8)]
        for cj in range(8):
            for j in range(8):
                nc.tensor.matmul(psq[j][:], wq_sb[:, cj, 128 * j:128 * (j + 1)],
                                 xq[:, cj, :], start=(cj == 0), stop=(cj == 7))
        for j in range(8):
            cast(0, qt[0:64, 2 * j, :], psq[j][0:64, :])
            cast(1, qt[0:64, 2 * j + 1, :], psq[j][64:128, :])

        # ---- consts: bias tables + on-chip EW generation ----
        nc.sync.dma_start(biasall[:], p["biasall"].ap())
        dbase = xw.tile([128, 896], FP32, tag="xk")   # reuse xk's slot
        nc.sync.dma_start(dbase[:], p["dbase"].ap())
        for h in range(H):
            nc.scalar.activation(ew[:, h, :], dbase[:], Exp,
                                 scale=-float(SLOPES[h]))
        nc.vector.memset(vfull[:, :, :, 64:65], 1.0)

        # ---- V projection inputs ----
        xv = xw.tile([128, 8, T], BF16, tag="xv")
        wv_sb = wrot.tile([128, 8, D], BF16, tag="w")
        load_split(xv, wv_sb, "xv", "wv")

    # own V tiles (local kt 0..3), cj-outer over 8 live psum banks
    with tc.tile_pool(name="vop", bufs=1, space="PSUM") as vop:
        psv = [vop.tile([128, 512], FP32, tag=f"v{i}", name=f"psv{i}")
               for i in range(8)]
        for cj in range(8):
            for tc_i in range(4):
                for nh in range(2):
                    nc.tensor.matmul(psv[2 * tc_i + nh][:],
                                     xv[:, cj, 128 * tc_i:128 * (tc_i + 1)],
                                     wv_sb[:, cj, 512 * nh:512 * (nh + 1)],
                                     start=(cj == 0), stop=(cj == 7))
        for tc_i in range(4):
            for nh in range(2):
                cast(nh, vfull[:, tc_i, 8 * nh:8 * (nh + 1), 0:64],
                     psv[2 * tc_i + nh][:].rearrange("p (h d) -> p h d", h=8))

    # ---- own-block attention interleaved with the remaining 12 V tiles ---
    with tc.tile_pool(name="vrp", bufs=2, space="PSUM") as vrp, \
         tc.tile_pool(name="stpo", bufs=2, space="PSUM") as stpo, \
         tc.tile_pool(name="oop", bufs=2, space="PSUM") as oop, \
         tc.tile_pool(name="expo", bufs=2) as epool_o:

        def v_rest_half(tc_i, half):
            ps = vrp.tile([128, 512], FP32, tag="vr")
            for cj in range(8):
                nc.tensor.matmul(ps[:],
                                 xv[:, cj, 128 * tc_i:128 * (tc_i + 1)],
                                 wv_sb[:, cj, 512 * half:512 * (half + 1)],
                                 start=(cj == 0), stop=(cj == 7))
            nc.vector.tensor_copy(
                vfull[:, tc_i, 8 * half:8 * (half + 1), 0:64],
                ps[:].rearrange("p (h d) -> p h d", h=8))

        vrest = [(4 + i, hf) for i in range(12) for hf in range(2)]
        vi = 0
        for h in range(H):
            oo = oop.tile([65, QS], FP32, tag="oo")
            sts, es = [], []
            for g in range(2):
                stps = stpo.tile([128, 2 * QS], FP32, tag="st")
                for j in range(2):
                    kt = 2 * g + j
                    nc.tensor.matmul(stps[:, QS * j:QS * (j + 1)],
                                     klocal[:, h, 128 * kt:128 * (kt + 1)],
                                     qt[0:64, h, :], start=True, stop=True)
                sts.append(stps)
            # 1-2 V half-tiles between scores and AV keep the PE busy while
            # the exp/window chain catches up
            for _ in range(2):
                if vi < len(vrest):
                    v_rest_half(*vrest[vi]); vi += 1
            for g in range(2):
                e = epool_o.tile([128, 2 * QS], BF16, tag="e")
                nc.scalar.activation(e[:], sts[g][:], Exp,
                                     bias=biasall[:, 8 * h:8 * h + 1],
                                     scale=1.0)
                for j in range(2):
                    kt = 2 * g + j
                    nc.vector.tensor_mul(e[:, QS * j:QS * (j + 1)],
                                         e[:, QS * j:QS * (j + 1)],
                                         ew[:, h, 384 - 128 * kt:896 - 128 * kt])
                    nc.tensor.matmul(oo[:], vfull[:, kt, h, :],
                                     e[:, QS * j:QS * (j + 1)],
                                     start=(kt == 0), stop=(kt == 3))
            nc.vector.tensor_copy(ot_own[:, h, :], oo[:])
        while vi < len(vrest):
            v_rest_half(*vrest[vi]); vi += 1

    pctx.close()   # xw/wrot SBUF freed for the rest-pass pools

    # --- per-core K rotation registers ---
    rvs = []
    for rl in range(4):
        reg = nc.sync.alloc_register(f"rot{rl}")
        nc.sync.reg_load(reg, rot_sb[0:1, rl:rl + 1])
        rvs.append(nc.sync.snap(reg, donate=True))
    agk_r = agk.ap().rearrange("r (h d) c -> r d h c", h=H)    # [4,64,H,QS]

    # ================= phase 2: rest pass (kt 4..15) ======================
    wo_sb = late.tile([128, 8, D], BF16)
    with tc.tile_pool(name="otn", bufs=1) as otpool, \
         tc.tile_pool(name="ktstream", bufs=4) as kts, \
         tc.tile_pool(name="exps", bufs=4) as epool, \
         tc.tile_pool(name="recip", bufs=3) as rpool, \
         tc.tile_pool(name="yout", bufs=2) as ypool, \
         tc.tile_pool(name="stps", bufs=3, space="PSUM") as stp, \
         tc.tile_pool(name="otps", bufs=2, space="PSUM") as otp:

        ot = otpool.tile([128, 8, QS], BF16)    # normalized O^T
        for hi, h in enumerate(HEAD_ORDER):
            kept = [g for g in range(6) if g not in SKIP_GROUPS[h]]
            kth = kts.tile([66, 3 * QS], BF16, tag="kth")
            for rl in range(1, 4):
                if all((kt - 4) // 2 in SKIP_GROUPS[h]
                       for kt in range(4 * rl, 4 * rl + 4)):
                    continue    # whole block below the mass threshold
                nc.sync.dma_start(kth[0:64, QS * (rl - 1):QS * rl],
                                  agk_r[bass.ds(rvs[rl], 1), :, h, :])
            nc.sync.dma_start(kth[64:66, :], p["srow"].ap()[h, :, QS:])

            otps = otp.tile([65, QS], FP32, tag="ot")
            for g in kept:
                stps = stp.tile([128, 2 * QS], FP32, tag="st")
                for j in range(2):
                    kt = 4 + 2 * g + j
                    nc.tensor.matmul(stps[:, QS * j:QS * (j + 1)],
                                     kth[:, 128 * (kt - 4):128 * (kt - 3)],
                                     qt[:, h, :], start=True, stop=True)
                e = epool.tile([128, 2 * QS], BF16, tag="e")
                nc.scalar.activation(e[:], stps[:], Exp,
                                     bias=biasall[:, 8 * h + 1 + g:8 * h + 2 + g],
                                     scale=1.0)
                for j in range(2):
                    kt = 4 + 2 * g + j
                    nc.tensor.matmul(otps[:], vfull[:, kt, h, :],
                                     e[:, QS * j:QS * (j + 1)],
                                     start=(g == kept[0] and j == 0),
                                     stop=(g == kept[-1] and j == 1))
            nc.vector.tensor_add(otps[:], otps[:], ot_own[:, h, :])
            zrow = rpool.tile([1, QS], FP32, tag="zrow")
            nc.vector.tensor_copy(zrow[:], otps[64:65, :])
            rec = rpool.tile([1, QS], FP32, tag="rec")
            # approx recip needs a partition-0 fp32 source; ~51 ULP is plenty
            nc.vector.reciprocal_approx_fast(rec[:], zrow[:])
            if DEBUG_DUMP:
                nc.sync.dma_start(p["d_z"].ap()[:, h, :], zrow[:])
                nc.sync.dma_start(p["d_rec"].ap()[:, h, :], rec[:])
            bcs = rpool.tile([64, QS], FP32, tag="bcs")
            nc.gpsimd.partition_broadcast(bcs[:], rec[:])
            nc.vector.tensor_mul(ot[64 * (h % 2):64 * (h % 2) + 64, h // 2, :],
                                 otps[0:64, :], bcs[:])
            if hi == 0:
                # prefetch the out-proj weight while the DMA queue is idle
                nc.sync.dma_start(wo_sb[:], p["wo"].ap().rearrange(
                    "(j p) c -> p j c", p=128))

        # --- out-projection (reuses the "ot" psum slots) ---
        for tc_i in range(4):
            y = ypool.tile([128, D], FP32, tag="y")
            for nh in range(2):
                ps = otp.tile([128, 512], FP32, tag="ot", name=f"ops{tc_i}{nh}")
                for j in range(8):
                    nc.tensor.matmul(ps[:], ot[:, j, 128 * tc_i:128 * (tc_i + 1)],
                                     wo_sb[:, j, 512 * nh:512 * (nh + 1)],
                                     start=(j == 0), stop=(j == 7))
                cast(nh, y[:, 512 * nh:512 * (nh + 1)], ps[:])
            nc.sync.dma_start(p["out"].ap()[128 * tc_i:128 * (tc_i + 1), :], y[:])

        if DEBUG_DUMP:
            nc.sync.dma_start(p["d_klocal"].ap(), klocal[:])
            nc.sync.dma_start(p["d_qt"].ap(), qt[:])
            nc.sync.dma_start(p["d_vfull"].ap(), vfull[:])
            nc.sync.dma_start(p["d_otown"].ap(), ot_own[:])
            nc.sync.dma_start(p["d_ew"].ap(), ew[:])
            nc.sync.dma_start(p["d_ot"].ap(), ot[:])

    ctx.close()


# --------------------------------------------------------------------------
# host side
# --------------------------------------------------------------------------

def _prep_core_inputs(inputs, c):
    b, s = divmod(c, 4)
    q0 = QS * s
    sl = slice(q0, q0 + QS)
    f32 = np.float32

    for bn in ("bq", "bk", "bv", "bo"):
        assert not np.any(np.asarray(inputs[bn])), \
            f"nonzero {bn} not supported by this kernel build"

    def tr(x):
        return np.ascontiguousarray(np.asarray(x, f32).T)

    xv_rot = np.roll(tr(inputs["value"][b]), -q0, axis=1)  # local coords
    m = {
        "xq": tr(inputs["query"][b][sl]).astype(BF16_NP),
        "xk": tr(inputs["key"][b][sl]).astype(BF16_NP),
        "xv": xv_rot.astype(BF16_NP),
        "wq": (np.asarray(inputs["Wq"], f32) * HD ** -0.5).astype(BF16_NP),
        "wk": np.asarray(inputs["Wk"], f32).astype(BF16_NP),
        "wv": np.asarray(inputs["Wv"], f32).astype(BF16_NP),
        "wo": np.asarray(inputs["Wo"], f32).astype(BF16_NP),
    }

    qlo = np.zeros((2, H, QS), f32)
    qlo[0] = (np.arange(QS, dtype=f32) - 256.0)[None, :]
    qlo[1] = (128.0 * SLOPES)[:, None]
    m["qlo"] = qlo.astype(BF16_NP)

    # local k coords; wrap where k_local >= T - q0 (512-aligned)
    kloc = np.arange(T)
    wrap = kloc >= (T - q0) if q0 > 0 else np.zeros(T, bool)
    ktv = kloc // 128
    srow = np.zeros((H, 2, T), f32)
    biasall = np.zeros((128, H, 8), f32)
    pvec = np.arange(128, dtype=f32)
    for h in range(H):
        sh = SLOPES[h]
        # row 0: coefficient of (q_lo - 256); row 1: coefficient of 128*s
        srow[h, 0, 512:] = np.where(wrap[512:], -sh, sh)
        srow[h, 1, 512:] = np.where(wrap[512:], ktv[512:] - 18.0,
                                    2.0 - ktv[512:])
        for g in range(6):
            kt = 4 + 2 * g
            biasall[:, h, 1 + g] = (sh * pvec) if wrap[128 * kt] else (-sh * pvec)
    m["srow"] = srow.astype(BF16_NP)
    m["biasall"] = biasall.reshape(128, H * 8)

    col = np.arange(896, dtype=f32)
    m["dbase"] = np.abs(pvec[:, None] - col[None, :] + 384.0).astype(f32)

    m["rotidx"] = np.asarray([[(rl + s) % 4 for rl in range(4)]], np.int32)
    return m


_NC_CACHE = {}


def _get_nc():
    if "nc" not in _NC_CACHE:
        _NC_CACHE["nc"] = _build_graph()
    return _NC_CACHE["nc"]


def run(inputs, trace=False, trace_kwargs=None):
    nc = _get_nc()
    in_maps = [_prep_core_inputs(inputs, c) for c in range(NCORES)]
    res = run_bass_kernel_spmd(nc, in_maps, list(range(NCORES)),
                               trace=trace, **(trace_kwargs or {}))
    out = np.empty((B, T, D), np.float32)
    for c in range(NCORES):
        b, s = divmod(c, 4)
        out[b, QS * s:QS * (s + 1), :] = res.results[c]["out"]
    return out, res


def kernel(**inputs):
    return run(inputs)[0]
